# revision 1
# baseline (speedup 1.0000x reference)
"""CommNet actor kernel for Trainium2 (Bass/Tile), 8-core data-parallel.

Math (per sample, A=32 agents, D=128 obs, H=64 hidden, NA=16 actions):
    h   = tanh(obs @ enc_w + enc_b)
    2 rounds of:  messages = h @ comm_w + comm_b
                  received = (sum_agents(messages) - messages) / (A-1)
                  h = tanh([h, received] @ upd_w + upd_b)
    out = tanh(h @ dec_w1 + dec_b1) @ dec_w2 + dec_b2

The round is folded on the host into  h' = tanh(h @ W1 + s @ W2 + b)  where
s = sum_agents(h), W1 = U_top - comm_w @ U_bot / (A-1), W2 = comm_w @ U_bot / (A-1),
b = comm_b @ U_bot + upd_b   (U_top/U_bot = upd_w[:H], upd_w[H:]).

Device layout: feature-major activations [feat, tok].  Each "unit" is 2048
tokens as one [128, 1024] tile: column t holds features of token t (T0) on
partitions 0:64 and of token 1024+t (T1) on partitions 64:128.  ALL matmuls
are bf16 (1 cyc/col moving operand; fp32/f32r cost 2x).  The two partition
halves use block-diagonal weights kron(I2, W); the encoder stacks halves via
a zero-padded lhsT accumulation pair.  tanh/reduce process a unit in single
[128, 1024] instructions (full 128-lane utilization).

dec2 writes logits for all 4 quarter-chunks of a unit into one [128, 256]
psum region using 4 zero-padded [128, 128] lhsT variants (block j holds
kron(I2, dec_w2) at columns 32j:32j+32), so every matmul writes the full
partition range and the PSUM accumulation-group semantics stay standard.
The output copy is then FD=256 over 128 lanes (3x cheaper than [32, 1024])
and the store is bf16 (half the HBM write traffic), decoded on the host.

Units are emitted in interleaved groups of four (software pipelining at the
emission level): one unit's matmuls fill the PE gaps left by another unit's
tanh/reduce stages, keeping the PE HAM-warm.  obs is pre-transposed on the
host into the exact feature-major bf16 DMA layout (one 512 KB load/unit).
"""

import numpy as np
from contextlib import ExitStack

import concourse.bass as bass
import concourse.bacc as bacc
import concourse.tile as tile
from concourse import mybir
from concourse.bass_utils import run_bass_kernel_spmd

# Problem constants
B, A, D, H, NA = 16384, 32, 128, 64, 16
R = 2
NCORES = 8
S_CORE = B // NCORES          # 2048 samples per core
TOK = S_CORE * A              # 65536 tokens per core
HALF_TOK = 1024               # tokens per half-unit
UNIT_TOK = 2 * HALF_TOK       # 2048 tokens per unit
NU = TOK // UNIT_TOK          # 32 units per core
SAMP_HALF = HALF_TOK // A     # 32 samples per half-unit column space
OCH = 256                     # dec2 output columns (quarter-unit chunks)
FP = mybir.dt.float32
BF = mybir.dt.bfloat16
TANH = mybir.ActivationFunctionType.Tanh

# wpack16 (bf16) column layout
_C_ENC = 0              # enc_w                  [128, 64]   (T0 encoder)
_C_ENCP = 64            # [0 | enc_w]            [128, 128]  (T1 encoder, zero-pad)
_C_W1 = (192, 448)      # kron(I2, W1_r)         [128, 128] per round
_C_W2 = (320, 576)      # kron(I2, W2_r)         [128, 128] per round
_C_D1 = 704             # kron(I2, dec_w1)       [128, 128]
_C_D2 = 832             # 4x zero-padded dec2    [128, 128] per chunk j
NW16 = _C_D2 + 4 * 128
NWB = 4                 # fp32 bias cols: enc, r0, r1, dec1 (each stacked [b; b])


def build_body(ctx, tc, obs_t, wpack16, wb, out, n_units):
    nc = tc.nc
    wpool = ctx.enter_context(tc.tile_pool(name="w", bufs=1))
    obs_pool = ctx.enter_context(tc.tile_pool(name="obs", bufs=9))
    h_pool = ctx.enter_context(tc.tile_pool(name="h", bufs=24))
    s_pool = ctx.enter_context(tc.tile_pool(name="s", bufs=16))
    osb_pool = ctx.enter_context(tc.tile_pool(name="osb", bufs=8))
    ps_pool = ctx.enter_context(tc.tile_pool(name="ps", bufs=4, space="PSUM"))

    w16 = wpool.tile([D, NW16], BF)
    nc.sync.dma_start(out=w16[:], in_=wpack16)
    wbt = wpool.tile([D, NWB], FP)
    nc.sync.dma_start(out=wbt[:], in_=wb)

    w_enc = w16[:, _C_ENC : _C_ENC + 64]
    w_encp = w16[:, _C_ENCP : _C_ENCP + 128]
    w1 = [w16[:, _C_W1[r] : _C_W1[r] + 128] for r in range(R)]
    w2 = [w16[:, _C_W2[r] : _C_W2[r] + 128] for r in range(R)]
    w_d1 = w16[:, _C_D1 : _C_D1 + 128]
    w_d2j = [w16[:, _C_D2 + 128 * j : _C_D2 + 128 * (j + 1)] for j in range(4)]
    b_enc = wbt[:, 0:1]
    b_r = [wbt[:, 1 + r : 2 + r] for r in range(R)]
    b_d1 = wbt[:, 3:4]

    c0 = slice(0, 512)
    c1 = slice(512, 1024)

    def emit_load(u):
        obs = obs_pool.tile([D, UNIT_TOK], BF, tag="obs")
        nc.sync.dma_start(out=obs[:], in_=obs_t[u])
        return obs

    def emit_enc_mms(obs):
        # obs cols: h*1024 + t.  T1 half via zero-padded [0|enc_w] (start),
        # then T0 accumulates into partitions 0:64 (stop).
        ps = ps_pool.tile([128, HALF_TOK], FP, tag="ps")
        for cs in (c0, c1):
            nc.tensor.matmul(ps[:, cs],
                             lhsT=w_encp,
                             rhs=obs[:, 1024 + cs.start : 1024 + cs.stop],
                             start=True, stop=False, skip_group_check=True)
        for cs in (c0, c1):
            nc.tensor.matmul(ps[0:64, cs], lhsT=w_enc, rhs=obs[:, cs],
                             start=False, stop=True, skip_group_check=True)
        return ps

    def emit_tanh(ps, bias):
        hh = h_pool.tile([128, HALF_TOK], BF, tag="h")
        nc.scalar.activation(hh[:], ps[:], TANH, bias=bias)
        return hh

    def emit_reduce(hh):
        s = s_pool.tile([128, SAMP_HALF], BF, tag="s")
        with nc.allow_low_precision(reason="bf16 agent-sum; tolerance 2e-2"):
            nc.vector.reduce_sum(
                out=s[:],
                in_=hh.rearrange("p (g a) -> p g a", a=A),
                axis=mybir.AxisListType.X,
            )
        return s

    def emit_round_mms(r, hh, s):
        ns = SAMP_HALF // 2  # samples per 512-token column block
        ps = ps_pool.tile([128, HALF_TOK], FP, tag="ps")
        for cs in (c0, c1):
            nc.tensor.matmul(ps[:, cs], lhsT=w1[r], rhs=hh[:, cs],
                             start=True, stop=False, skip_group_check=True)
        for b, cs in ((0, c0), (1, c1)):
            sb = s[:, b * ns : (b + 1) * ns].unsqueeze(2).broadcast_to(
                [128, ns, A]
            )
            nc.tensor.matmul(ps[:, cs], lhsT=w2[r], rhs=sb,
                             start=False, stop=True, skip_group_check=True)
        return ps

    def emit_dec1_mms(hh):
        ps = ps_pool.tile([128, HALF_TOK], FP, tag="ps")
        for cs in (c0, c1):
            nc.tensor.matmul(ps[:, cs], lhsT=w_d1, rhs=hh[:, cs],
                             skip_group_check=True)
        return ps

    def emit_dec2_mms(pre):
        # 4 quarter-chunks accumulate into [128, 256]: chunk j's zero-padded
        # lhsT routes its logits to partitions 32j:32j+32, writing the full
        # partition range each time (rows outside the block are zeros), so
        # accumulation-group semantics stay standard and the output copy is
        # FD=256 over all 128 lanes.
        po = ps_pool.tile([128, HALF_TOK], FP, tag="ps")
        for j in range(4):
            nc.tensor.matmul(po[:, 0:OCH], lhsT=w_d2j[j],
                             rhs=pre[:, OCH * j : OCH * (j + 1)],
                             start=(j == 0), stop=(j == 3),
                             skip_group_check=True)
        return po

    def emit_out(u, po):
        osb = osb_pool.tile([128, OCH], BF, tag="osb")
        nc.vector.tensor_copy(osb[:], po[:, 0:OCH])
        nc.sync.dma_start(out=out[u], in_=osb[:])

    # -- warmup: a dummy activation on the (already loading) weight tile
    # triggers the tanh ACT_TABLE_LOAD during the first obs DMAs instead of
    # serializing it in front of the first real tanh.
    dummy_out = wpool.tile([128, 8], BF)
    nc.scalar.activation(dummy_out[:], w16[:, 0:8], TANH)

    # -- software-pipelined emission over groups of four units: group g+1's
    # encoder matmuls and tanh are emitted before group g's dec2/out so the
    # PE fills the scalar engine's group-boundary latency.
    groups = [list(range(u0, u0 + 4)) for u0 in range(0, n_units, 4)]
    obs = {u: emit_load(u) for u in groups[0]}

    def emit_enc_stage(grp):
        ps = [emit_enc_mms(obs.pop(u)) for u in grp]
        return [emit_tanh(p, b_enc) for p in ps]

    hs = {0: emit_enc_stage(groups[0])}
    for u in groups[1][:] if len(groups) > 1 else []:
        obs[u] = emit_load(u)
    for gi, grp in enumerate(groups):
        cur = hs.pop(gi)
        if gi + 2 < len(groups):
            for u in groups[gi + 2]:
                obs[u] = emit_load(u)
        for r in range(R):
            ss = [emit_reduce(hh) for hh in cur]
            ps = [emit_round_mms(r, hh, s) for hh, s in zip(cur, ss)]
            cur = [emit_tanh(p, b_r[r]) for p in ps]
        ps = [emit_dec1_mms(hh) for hh in cur]
        pres = [emit_tanh(p, b_d1) for p in ps]
        if gi + 1 < len(groups):
            hs[gi + 1] = emit_enc_stage(groups[gi + 1])
        pos = [emit_dec2_mms(pre) for pre in pres]
        for u, po in zip(grp, pos):
            emit_out(u, po)


def build_nc(n_units=NU):
    nc = bacc.Bacc(None, target_bir_lowering=False, debug=False)
    obs_t = nc.declare_dram_parameter(
        "obs_t", [n_units, D, UNIT_TOK], BF, isOutput=False
    )
    wpack16 = nc.declare_dram_parameter("wpack16", [D, NW16], BF, isOutput=False)
    wb = nc.declare_dram_parameter("wb", [D, NWB], FP, isOutput=False)
    out = nc.declare_dram_parameter(
        "out", [n_units, 128, OCH], BF, isOutput=True
    )
    with tile.TileContext(nc) as tc:
        with ExitStack() as ctx:
            build_body(ctx, tc, obs_t[:], wpack16[:], wb[:], out[:], n_units)
    nc.compile()
    return nc


def fold_weights(enc_w, enc_b, comm_w, comm_b, upd_w, upd_b, dec_w1, dec_b1, dec_w2):
    """Host-side algebraic fold + packing (float64 math)."""
    import ml_dtypes

    f8 = np.float64
    denom = f8(max(A - 1, 1))
    wb = np.zeros((D, NWB), np.float32)
    wpack16 = np.zeros((D, NW16), np.float32)

    def bd(Wm):  # kron(I2, W) for [64, x] -> [128, 2x]
        Wm = np.asarray(Wm, np.float32)
        k, m = Wm.shape
        o = np.zeros((2 * k, 2 * m), np.float32)
        o[:k, :m] = Wm
        o[k:, m:] = Wm
        return o

    wpack16[:, _C_ENC : _C_ENC + 64] = np.asarray(enc_w, np.float32)
    wpack16[:, _C_ENCP + 64 : _C_ENCP + 128] = np.asarray(enc_w, np.float32)
    for r in range(R):
        C = np.asarray(comm_w[r], f8)
        Ut = np.asarray(upd_w[r][:H], f8)
        Ub = np.asarray(upd_w[r][H:], f8)
        G = C @ Ub / denom
        W1 = (Ut - G).astype(np.float32)
        W2 = G.astype(np.float32)
        br = (np.asarray(comm_b[r], f8) @ Ub + np.asarray(upd_b[r], f8)).astype(
            np.float32
        )
        wpack16[:, _C_W1[r] : _C_W1[r] + 128] = bd(W1)
        wpack16[:, _C_W2[r] : _C_W2[r] + 128] = bd(W2)
        wb[0:64, 1 + r] = br
        wb[64:128, 1 + r] = br
    wpack16[:, _C_D1 : _C_D1 + 128] = bd(dec_w1)
    d2 = bd(dec_w2)  # [128, 32]
    for j in range(4):
        wpack16[:, _C_D2 + 128 * j + 32 * j : _C_D2 + 128 * j + 32 * (j + 1)] = d2
    be = np.asarray(enc_b, np.float32)
    wb[0:64, 0] = be
    wb[64:128, 0] = be
    bd1 = np.asarray(dec_b1, np.float32)
    wb[0:64, 3] = bd1
    wb[64:128, 3] = bd1
    return wpack16.astype(ml_dtypes.bfloat16), wb


def prep_obs(obs):
    """[B, A, D] -> [NCORES, NU, D, 2048] feature-major bf16."""
    import ml_dtypes

    obs4 = np.asarray(obs, np.float32).reshape(NCORES, NU, UNIT_TOK, D)
    return np.ascontiguousarray(
        obs4.transpose(0, 1, 3, 2).astype(ml_dtypes.bfloat16)
    )


_NC_CACHE = {}


def _get_nc(n_units=NU):
    if n_units not in _NC_CACHE:
        _NC_CACHE[n_units] = build_nc(n_units)
    return _NC_CACHE[n_units]


def kernel(
    obs,
    enc_w,
    enc_b,
    comm_w,
    comm_b,
    upd_w,
    upd_b,
    dec_w1,
    dec_b1,
    dec_w2,
    dec_b2,
    _trace=False,
    _trace_kwargs=None,
):
    wpack16, wb = fold_weights(
        enc_w, enc_b, comm_w, comm_b, upd_w, upd_b, dec_w1, dec_b1, dec_w2
    )
    obs_t = prep_obs(obs)
    nc = _get_nc()
    in_maps = [
        {"obs_t": obs_t[i], "wpack16": wpack16, "wb": wb}
        for i in range(NCORES)
    ]
    res = run_bass_kernel_spmd(
        nc,
        in_maps,
        core_ids=list(range(NCORES)),
        trace=_trace,
        **(_trace_kwargs or {}),
    )
    outs = np.stack([res.results[i]["out"] for i in range(NCORES)])
    # out[u, 32j+16h+a, c] = logits(tok = u*2048 + h*1024 + 256j + c, action a)
    o = np.asarray(outs, np.float32).reshape(NCORES, NU, 4, 2, NA, OCH)
    o = o.transpose(0, 1, 3, 2, 5, 4)  # -> [core, u, h, j, c, a]
    logits = o.reshape(B, A, NA) + np.asarray(dec_b2, np.float32)[None, None, :]
    if _trace:
        return logits.astype(np.float32), res
    return logits.astype(np.float32)

